# revision 18
# baseline (speedup 1.0000x reference)
"""Cross-attention layer with 3D RoPE on 8 Trainium2 NeuronCores.

Sharding: core c -> (batch b = c//2, head-group hg = c%2 of 4 heads).
Each core computes its batch's partial output projection for its 4 heads;
the host sums the two half-partials per batch and adds the output bias.

All device tensors are channels-major ([d, tokens]); the host pre-transposes
inputs and weights with numpy so the kernel needs no on-device transposes.

Matmul operands are bf16 (fast weight load, half DMA/SBUF); accumulation
stays fp32 in PSUM.  The attention inner loop is software-pipelined so the
PE never head-of-line blocks on the Scalar-engine exp.
"""

import sys

import numpy as np

try:
    import concourse.bass as bass
except ImportError:  # fresh-dir grading: container repo path
    sys.path.insert(0, "/opt/trn_rl_repo")
    import concourse.bass as bass

import ml_dtypes
import concourse.tile as tile
from concourse import bacc, bass_utils, mybir

F32 = mybir.dt.float32
MM_DT = mybir.dt.float32r  # full-rate reduced-precision fp32 matmul mode
BF16 = mybir.dt.bfloat16
NP_BF16 = ml_dtypes.bfloat16

B, Lq, Lk, D, H = 4, 1024, 4096, 768, 8
HD = 96            # head dim
HL = 4             # heads per core
DL = HL * HD       # 384 local d per core
N_CORES = 8
SCALE = 1.0 / float(np.sqrt(np.float32(HD)))
PI = float(np.pi)


def build_program():
    nc = bacc.Bacc("TRN2", target_bir_lowering=False, debug=False)
    AF = mybir.ActivationFunctionType
    ALU = mybir.AluOpType

    # DRAM I/O (per-core shapes)
    dqT = nc.dram_tensor("queryT", [D, Lq], BF16, kind="ExternalInput").ap()
    dkT = nc.dram_tensor("keyT", [D, Lk], BF16, kind="ExternalInput").ap()
    dvT = nc.dram_tensor("valueT", [D, Lk], BF16, kind="ExternalInput").ap()
    dcq = nc.dram_tensor("cqT", [8, Lq], MM_DT, kind="ExternalInput").ap()
    dck = nc.dram_tensor("ckT", [8, Lk], MM_DT, kind="ExternalInput").ap()
    dwq = nc.dram_tensor("wqT", [D, DL], BF16, kind="ExternalInput").ap()
    dwk = nc.dram_tensor("wkT", [D, DL], BF16, kind="ExternalInput").ap()
    dwv = nc.dram_tensor("wvT", [D, DL], BF16, kind="ExternalInput").ap()
    dwo = nc.dram_tensor("woT", [4 * 128, D], BF16, kind="ExternalInput").ap()
    dbq = nc.dram_tensor("bqh", [128, HL], F32, kind="ExternalInput").ap()
    dbk = nc.dram_tensor("bkh", [128, HL], F32, kind="ExternalInput").ap()
    dbv = nc.dram_tensor("bvb", [128, DL], F32, kind="ExternalInput").ap()
    divf = nc.dram_tensor("ivf", [8, HD], MM_DT, kind="ExternalInput").ap()
    dp96 = nc.dram_tensor("p96", [128, HD], MM_DT, kind="ExternalInput").ap()
    done96 = nc.dram_tensor("one96", [1, HD], MM_DT, kind="ExternalInput").ap()
    dout = nc.dram_tensor("outT", [D, Lq], F32, kind="ExternalOutput").ap()

    NC6 = D // 128     # 6 c-tiles of the contraction dim
    NJQ = Lq // 512    # 2 q token tiles
    NJK = Lk // 512    # 8 k token tiles
    NKT = Lk // 128    # 32 k tiles for attention
    GRP = 2            # score k-tiles per exp instruction

    with tile.TileContext(nc) as tc:
        from contextlib import ExitStack

        with ExitStack() as ctx:
            # ---- persistent tensors ----
            big = ctx.enter_context(tc.tile_pool(name="big", bufs=1))
            kT = big.tile([HD, HL, Lk], BF16, tag="kT")
            qT = big.tile([HD, HL, Lq], BF16, tag="qT")
            vsb = big.tile([128, NKT, HL, HD + 1], BF16, tag="vsb")
            o2n = big.tile([HD, HL, Lq], BF16, tag="o2n")
            cst = ctx.enter_context(tc.tile_pool(name="cst", bufs=1))
            ivf = cst.tile([8, HD], MM_DT, tag="ivf")
            p96 = cst.tile([128, HD], MM_DT, tag="p96")
            bqh = cst.tile([128, HL], F32, tag="bqh")
            bkh = cst.tile([128, HL], F32, tag="bkh")
            bvb = cst.tile([128, DL], F32, tag="bvb")
            one96 = cst.tile([1, HD], MM_DT, tag="one96")

            nc.scalar.dma_start(ivf[:], divf[:])
            nc.scalar.dma_start(p96[:], dp96[:])
            nc.scalar.dma_start(bqh[:], dbq[:])
            nc.scalar.dma_start(bkh[:], dbk[:])
            nc.scalar.dma_start(bvb[:], dbv[:])
            nc.scalar.dma_start(one96[:], done96[:])

            # ones column of v (softmax denominator accumulates in po[HD])
            nc.gpsimd.memset(vsb[:, :, :, HD : HD + 1], 1.0)

            # ---- q/k projections + rope ----
            # Two-stage software pipeline: the rope shuffle matmul (xs) lags
            # one head behind the projection (so the PE never waits on the
            # Scalar-engine xf copy), and the elementwise rope ops lag two
            # heads (so the xs PSUM slot never waits on the DVE FIFO).
            def proj_rope(src_dram, coords_dram, wT_tiles, bias_sb, dst, n_jt):
                pend_xs = [None]
                pend_alu = [None]

                def emit_xs(xf, sinD, cosD, out_ap):
                    xs = xs_ps.tile([HD, 512], F32, tag="xs")
                    nc.tensor.matmul(
                        xs[:], (p96[0:HD, :]), (xf[:]), start=True, stop=True
                    )
                    pend_alu[0] = (xf, xs, sinD, cosD, out_ap)

                def emit_alu(xf, xs, sinD, cosD, out_ap):
                    m2 = tmp.tile([HD, 512], F32, tag="m2")
                    nc.vector.tensor_mul(m2[:], xs[:], sinD[:])
                    m1 = tmp.tile([HD, 512], F32, tag="m1")
                    nc.gpsimd.tensor_mul(m1[:], xf[:].bitcast(F32), cosD[:])
                    nc.vector.tensor_add(out_ap, m1[:], m2[:])

                def step(nxt):
                    # advance the two-deep pipeline by one head
                    if pend_alu[0] is not None:
                        alu, pend_alu[0] = pend_alu[0], None
                    else:
                        alu = None
                    if pend_xs[0] is not None:
                        emit_xs(*pend_xs[0])  # sets pend_alu
                    pend_xs[0] = nxt
                    if alu is not None:
                        emit_alu(*alu)

                for jt in range(n_jt):
                    ts = slice(jt * 512, (jt + 1) * 512)
                    # theta -> wrapped -> sin/cos  (rows: axis*32 + half*16 + p)
                    cstg = cpool.tile([8, 512], MM_DT, tag="coords")
                    nc.sync.dma_start(cstg[:], coords_dram[:, ts])
                    stg = [stage.tile([128, 512], BF16, tag="stg", name=f"stg{c}") for c in range(NC6)]
                    for c in range(NC6):
                        nc.sync.dma_start(
                            stg[c][:], src_dram[c * 128 : (c + 1) * 128, ts]
                        )
                    th = th_ps.tile([HD, 512], F32, tag="th")
                    nc.tensor.matmul(th[:], ivf[:], cstg[:], start=True, stop=True)
                    ws = trig.tile([HD, 512], F32, tag="ws")
                    wc = trig.tile([HD, 512], F32, tag="wc")
                    nc.vector.add_range_wrap(ws[:], th[:], 0.0, PI, 2 * PI)
                    nc.vector.add_range_wrap(wc[:], th[:], PI / 2, PI, 2 * PI)
                    sinD = trig.tile([HD, 512], F32, tag="sin")
                    cosD = trig.tile([HD, 512], F32, tag="cos")
                    nc.scalar.activation(sinD[:], ws[:], AF.Sin)
                    nc.scalar.activation(cosD[:], wc[:], AF.Sin)
                    for h in range(HL):
                        ps = pj_ps.tile([HD, 512], F32, tag="pj")
                        for c in range(NC6):
                            nc.tensor.matmul(
                                ps[:],
                                (wT_tiles[c][:, h * HD : (h + 1) * HD]),
                                (stg[c][:]),
                                start=(c == 0),
                                stop=(c == NC6 - 1),
                            )
                        xf = xf_p.tile([HD, 512], MM_DT, tag="xf")
                        nc.scalar.activation(
                            xf[:], ps[:], AF.Identity, bias=bias_sb[0:HD, h : h + 1]
                        )
                        step((xf, sinD, cosD, dst[:, h, ts]))
                # drain the pipeline
                step(None)
                step(None)

            with ExitStack() as pctx:
                stage = pctx.enter_context(tc.tile_pool(name="stage", bufs=12))
                cpool = pctx.enter_context(tc.tile_pool(name="cpool", bufs=2))
                trig = pctx.enter_context(tc.tile_pool(name="trig", bufs=2))
                tmp = pctx.enter_context(tc.tile_pool(name="tmp", bufs=2))
                xf_p = pctx.enter_context(tc.tile_pool(name="xf", bufs=3))
                th_ps = pctx.enter_context(
                    tc.tile_pool(name="th_ps", bufs=1, space="PSUM")
                )
                pj_ps = pctx.enter_context(
                    tc.tile_pool(name="pj_ps", bufs=2, space="PSUM")
                )
                vp_ps = pctx.enter_context(
                    tc.tile_pool(name="vp_ps", bufs=2, space="PSUM")
                )
                xs_ps = pctx.enter_context(
                    tc.tile_pool(name="xs_ps", bufs=3, space="PSUM")
                )

                with ExitStack() as wctx:
                    wq_p = wctx.enter_context(tc.tile_pool(name="wq", bufs=NC6))
                    wqT = [wq_p.tile([128, DL], BF16, tag="wq", name=f"wq{c}") for c in range(NC6)]
                    for c in range(NC6):
                        nc.scalar.dma_start(wqT[c][:], dwq[c * 128 : (c + 1) * 128, :])
                    proj_rope(dqT, dcq, wqT, bqh, qT, NJQ)

                with ExitStack() as wctx:
                    wk_p = wctx.enter_context(tc.tile_pool(name="wk", bufs=NC6))
                    wkT = [wk_p.tile([128, DL], BF16, tag="wk", name=f"wk{c}") for c in range(NC6)]
                    for c in range(NC6):
                        nc.sync.dma_start(wkT[c][:], dwk[c * 128 : (c + 1) * 128, :])
                    proj_rope(dkT, dck, wkT, bkh, kT, NJK)

                # ---- v projection (token-major, with bias add) ----
                with ExitStack() as wctx:
                    wv_p = wctx.enter_context(tc.tile_pool(name="wv", bufs=NC6))
                    wvT = [wv_p.tile([128, DL], BF16, tag="wv", name=f"wv{c}") for c in range(NC6)]
                    for c in range(NC6):
                        nc.sync.dma_start(wvT[c][:], dwv[c * 128 : (c + 1) * 128, :])
                    for jt in range(NJK):
                        ts = slice(jt * 512, (jt + 1) * 512)
                        stg = [stage.tile([128, 512], BF16, tag="stg", name=f"stg{c}") for c in range(NC6)]
                        for c in range(NC6):
                            nc.sync.dma_start(
                                stg[c][:], dvT[c * 128 : (c + 1) * 128, ts]
                            )
                        for sub in range(4):
                            kt = jt * 4 + sub
                            ps = vp_ps.tile([128, DL], F32, tag="vps")
                            for c in range(NC6):
                                nc.tensor.matmul(
                                    ps[:],
                                    (stg[c][:, sub * 128 : (sub + 1) * 128]),
                                    (wvT[c][:]),
                                    start=(c == 0),
                                    stop=(c == NC6 - 1),
                                )
                            nc.vector.scalar_tensor_tensor(
                                vsb[:, kt, :, 0:HD],
                                ps[:].rearrange("p (h d) -> p h d", h=HL),
                                0.0,
                                bvb[:].rearrange("p (h d) -> p h d", h=HL),
                                ALU.bypass,
                                ALU.add,
                            )

            # ---- attention + output projection ----
            # Flat software pipeline over all (jq, h, group) units: scores+exp
            # run one group ahead of attn@V, with no drain at head/jq
            # boundaries.  The softmax-denominator broadcast matmul is
            # deferred a couple of groups so the PE never waits on the DVE
            # normalize chain; jq0's output projection is interleaved into
            # jq1's score stream.
            with ExitStack() as actx:
                wo_p = actx.enter_context(tc.tile_pool(name="wo", bufs=4))
                woT = [wo_p.tile([128, D], BF16, tag="wo", name=f"wo{t}") for t in range(4)]
                for ht in range(4):
                    nc.sync.dma_start(woT[ht][:], dwo[ht * 128 : (ht + 1) * 128, :])
                s_ps = actx.enter_context(tc.tile_pool(name="s_ps", bufs=2, space="PSUM"))
                po_ps = actx.enter_context(tc.tile_pool(name="po_ps", bufs=2, space="PSUM"))
                bc_ps = actx.enter_context(tc.tile_pool(name="bc_ps", bufs=2, space="PSUM"))
                pt_p = actx.enter_context(tc.tile_pool(name="pt", bufs=3))
                nz_p = actx.enter_context(tc.tile_pool(name="nz", bufs=3))

                NG = NKT // GRP
                units = [(jq, h) for jq in range(NJQ) for h in range(HL)]
                groups = [(u, g) for u in range(len(units)) for g in range(NG)]
                po_of = {}
                pt_of = {}
                outproj_pend = []

                def emit_scores(u, g):
                    jq, h = units[u]
                    qs = slice(jq * 512, (jq + 1) * 512)
                    if g == 0:
                        po_of[u] = po_ps.tile([HD + 1, 512], F32, tag="po", name=f"po{u}")
                    sg = s_ps.tile([128, GRP * 512], F32, tag="sg")
                    for i in range(GRP):
                        kt = g * GRP + i
                        nc.tensor.matmul(
                            sg[:, i * 512 : (i + 1) * 512],
                            (kT[:, h, kt * 128 : (kt + 1) * 128]),
                            (qT[:, h, qs]),
                            start=True,
                            stop=True,
                        )
                    pt = pt_p.tile([128, GRP * 512], BF16, tag="pt")
                    nc.scalar.activation(pt[:], sg[:], AF.Exp, scale=SCALE)
                    pt_of[(u, g)] = pt

                def emit_attnv(u, g):
                    jq, h = units[u]
                    po = po_of[u]
                    pt = pt_of.pop((u, g))
                    for i in range(GRP):
                        kt = g * GRP + i
                        nc.tensor.matmul(
                            po[:],
                            (vsb[:, kt, h, :]),
                            (pt[:, i * 512 : (i + 1) * 512]),
                            start=(kt == 0),
                            stop=(kt == NKT - 1),
                        )
                    if g == NG - 1:
                        # start of normalize chain (DVE only; PE-independent)
                        s1 = nz_p.tile([1, 512], F32, tag="s1")
                        nc.vector.tensor_copy(s1[:], po[HD : HD + 1, :])
                        s1r = nz_p.tile([1, 512], F32, tag="s1r")
                        nc.vector.reciprocal_approx_fast(s1r[:], s1[:])
                        s1m = nz_p.tile([1, 512], MM_DT, tag="s1m")
                        nc.vector.tensor_copy(s1m[:], s1r[:])
                        po_of[u] = (po, s1m)

                def emit_normtail(u):
                    jq, h = units[u]
                    qs = slice(jq * 512, (jq + 1) * 512)
                    po, s1m = po_of.pop(u)
                    bcps = bc_ps.tile([HD, 512], F32, tag="bc", name="bcps")
                    nc.tensor.matmul(bcps[:], one96[:], s1m[:], start=True, stop=True)
                    bc = nz_p.tile([HD, 512], F32, tag="bc")
                    nc.vector.tensor_copy(bc[:], bcps[:])
                    nc.vector.tensor_mul(o2n[:, h, qs], po[0:HD, :], bc[:])

                def emit_outproj_e(jq, e):
                    qs = slice(jq * 512, (jq + 1) * 512)
                    pf = bc_ps.tile([128, 512], F32, tag="bc", name="pf")
                    for ht in range(4):
                        nc.tensor.matmul(
                            pf[:],
                            (woT[ht][0:HD, e * 128 : (e + 1) * 128]),
                            (o2n[:, ht, qs]),
                            start=(ht == 0),
                            stop=(ht == 3),
                        )
                    osb = nz_p.tile([128, 512], F32, tag="osb")
                    nc.vector.tensor_copy(osb[:], pf[:])
                    nc.gpsimd.dma_start(dout[e * 128 : (e + 1) * 128, qs], osb[:])

                LAG_BC = 3  # groups between last attn@V and the bcast matmul
                for idx in range(len(groups) + 1 + LAG_BC):
                    if idx < len(groups):
                        emit_scores(*groups[idx])
                    if 1 <= idx <= len(groups):
                        u, g = groups[idx - 1]
                        emit_attnv(u, g)
                        jq, h = units[u]
                        # interleave previous jq's output projection
                        if outproj_pend and g % 5 == 4:
                            emit_outproj_e(*outproj_pend.pop(0))
                    if idx - 1 - LAG_BC >= 0 and idx - 1 - LAG_BC < len(groups):
                        u, g = groups[idx - 1 - LAG_BC]
                        if g == NG - 1:
                            emit_normtail(u)
                            jq, h = units[u]
                            if h == HL - 1:
                                outproj_pend.extend(
                                    (jq, e) for e in range(NC6)
                                )
                # drain remaining output projections (jq1 tail)
                for jq, e in outproj_pend:
                    emit_outproj_e(jq, e)

    nc.compile()
    return nc


def _host_prep(inputs):
    """Build per-core input maps (numpy, bf16 for matmul operands)."""
    q = np.ascontiguousarray(np.asarray(inputs["query"], np.float32))
    k = np.ascontiguousarray(np.asarray(inputs["key"], np.float32))
    v = np.ascontiguousarray(np.asarray(inputs["value"], np.float32))
    cq = np.asarray(inputs["coords_query"], np.float32)
    ck = np.asarray(inputs["coords_key"], np.float32)
    Wq = np.asarray(inputs["Wq"], np.float32)
    Wk = np.asarray(inputs["Wk"], np.float32)
    Wv = np.asarray(inputs["Wv"], np.float32)
    Wo = np.asarray(inputs["Wo"], np.float32)
    bq = np.asarray(inputs["bq"], np.float32)
    bk = np.asarray(inputs["bk"], np.float32)
    bv = np.asarray(inputs["bv"], np.float32)

    inv_freq = (
        1.0 / (10000.0 ** (np.arange(16, dtype=np.float32) / np.float32(16.0)))
    ).astype(np.float32)
    ivf = np.zeros((8, HD), np.float32)
    for a in range(3):
        for h2 in range(2):
            ivf[a, a * 32 + h2 * 16 : a * 32 + h2 * 16 + 16] = inv_freq
    p96 = np.zeros((128, HD), np.float32)
    for a in range(3):
        for j in range(16):
            p96[a * 32 + 16 + j, a * 32 + j] = -1.0
            p96[a * 32 + j, a * 32 + 16 + j] = 1.0

    def pad_coords(c):  # [L, 3] -> [8, L]
        out = np.zeros((8, c.shape[0]), np.float32)
        out[:3] = c.T
        return out

    in_maps = []
    for c in range(N_CORES):
        b, hg = c // 2, c % 2
        dsl = slice(hg * DL, (hg + 1) * DL)
        wo_pad = np.zeros((4 * 128, D), np.float32)
        woT_full = np.ascontiguousarray(Wo.T)
        for ht in range(4):
            wo_pad[ht * 128 : ht * 128 + HD, :] = woT_full[
                hg * DL + ht * HD : hg * DL + (ht + 1) * HD, :
            ]
        bqh = np.zeros((128, HL), np.float32)
        bkh = np.zeros((128, HL), np.float32)
        for h in range(HL):
            bqh[:HD, h] = bq[hg * DL + h * HD : hg * DL + (h + 1) * HD]
            bkh[:HD, h] = bk[hg * DL + h * HD : hg * DL + (h + 1) * HD]
        bvb = np.tile(bv[dsl][None, :], (128, 1)).astype(np.float32)
        in_maps.append(
            {
                "queryT": np.ascontiguousarray(q[b].T).astype(NP_BF16),
                "keyT": np.ascontiguousarray(k[b].T).astype(NP_BF16),
                "valueT": np.ascontiguousarray(v[b].T).astype(NP_BF16),
                "cqT": pad_coords(cq[b]),
                "ckT": pad_coords(ck[b]),
                "wqT": np.ascontiguousarray(Wq[dsl, :].T).astype(NP_BF16),
                "wkT": np.ascontiguousarray(Wk[dsl, :].T).astype(NP_BF16),
                "wvT": np.ascontiguousarray(Wv[dsl, :].T).astype(NP_BF16),
                "woT": wo_pad.astype(NP_BF16),
                "bqh": bqh,
                "bkh": bkh,
                "bvb": bvb,
                "ivf": ivf,
                "p96": p96,
                "one96": np.ones((1, HD), np.float32),
            }
        )
    return in_maps


def _run(inputs, trace=False):
    nc = build_program()
    in_maps = _host_prep(inputs)
    res = bass_utils.run_bass_kernel_spmd(
        nc, in_maps, core_ids=list(range(N_CORES)), trace=trace
    )
    bo = np.asarray(inputs["bo"], np.float32)
    out = np.empty((B, Lq, D), np.float32)
    for b in range(B):
        acc = res.results[2 * b]["outT"] + res.results[2 * b + 1]["outT"]
        out[b] = acc.T + bo
    return out, res


def kernel(**inputs) -> np.ndarray:
    out, _ = _run(inputs, trace=False)
    return out


# revision 21
# speedup vs baseline: 1.2682x; 1.2682x over previous
"""Cross-attention layer with 3D RoPE on 8 Trainium2 NeuronCores.

Sharding: core c -> (batch b = c//2, head-group hg = c%2 of 4 heads).
Each core computes its batch's partial output projection for its 4 heads;
the host sums the two half-partials per batch and adds the output bias.

All device tensors are channels-major ([d, tokens]); the host pre-transposes
inputs and weights with numpy so the kernel needs no on-device transposes.

Matmul operands are bf16 (fast weight load, half DMA/SBUF); accumulation
stays fp32 in PSUM.  The attention inner loop is software-pipelined so the
PE never head-of-line blocks on the Scalar-engine exp.
"""

import sys

import numpy as np

try:
    import concourse.bass as bass
except ImportError:  # fresh-dir grading: container repo path
    sys.path.insert(0, "/opt/trn_rl_repo")
    import concourse.bass as bass

import ml_dtypes
import concourse.tile as tile
from concourse import bacc, bass_utils, mybir

F32 = mybir.dt.float32
MM_DT = mybir.dt.float32r  # full-rate reduced-precision fp32 matmul mode
BF16 = mybir.dt.bfloat16
NP_BF16 = ml_dtypes.bfloat16

B, Lq, Lk, D, H = 4, 1024, 4096, 768, 8
HD = 96            # head dim
HL = 4             # heads per core
DL = HL * HD       # 384 local d per core
N_CORES = 8
SCALE = 1.0 / float(np.sqrt(np.float32(HD)))
PI = float(np.pi)


def build_program():
    nc = bacc.Bacc("TRN2", target_bir_lowering=False, debug=False)
    AF = mybir.ActivationFunctionType
    ALU = mybir.AluOpType

    # DRAM I/O (per-core shapes)
    dqT = nc.dram_tensor("queryT", [D, Lq], BF16, kind="ExternalInput").ap()
    dkT = nc.dram_tensor("keyT", [D, Lk], BF16, kind="ExternalInput").ap()
    dvT = nc.dram_tensor("valueT", [D, Lk], BF16, kind="ExternalInput").ap()
    dsq = nc.dram_tensor("sinq", [HD, Lq], F32, kind="ExternalInput").ap()
    dcq = nc.dram_tensor("cosq", [HD, Lq], BF16, kind="ExternalInput").ap()
    dsk = nc.dram_tensor("sink", [HD, Lk], F32, kind="ExternalInput").ap()
    dck = nc.dram_tensor("cosk", [HD, Lk], BF16, kind="ExternalInput").ap()
    dwq = nc.dram_tensor("wqT", [D, DL], BF16, kind="ExternalInput").ap()
    dwk = nc.dram_tensor("wkT", [D, DL], BF16, kind="ExternalInput").ap()
    dwv = nc.dram_tensor("wvT", [D, DL], BF16, kind="ExternalInput").ap()
    dwo = nc.dram_tensor("woT", [4 * 128, D], BF16, kind="ExternalInput").ap()
    dbq = nc.dram_tensor("bqh", [128, HL], F32, kind="ExternalInput").ap()
    dbk = nc.dram_tensor("bkh", [128, HL], F32, kind="ExternalInput").ap()
    dbv = nc.dram_tensor("bvb", [128, DL], F32, kind="ExternalInput").ap()
    dp96 = nc.dram_tensor("p96", [128, HD], BF16, kind="ExternalInput").ap()
    done96 = nc.dram_tensor("one96", [1, HD], MM_DT, kind="ExternalInput").ap()
    dout = nc.dram_tensor("outT", [D, Lq], F32, kind="ExternalOutput").ap()

    NC6 = D // 128     # 6 c-tiles of the contraction dim
    NJQ = Lq // 512    # 2 q token tiles
    NJK = Lk // 512    # 8 k token tiles
    NKT = Lk // 128    # 32 k tiles for attention
    GRP = 2            # score k-tiles per exp instruction

    with tile.TileContext(nc) as tc:
        from contextlib import ExitStack

        with ExitStack() as ctx:
            # ---- persistent tensors ----
            big = ctx.enter_context(tc.tile_pool(name="big", bufs=1))
            kT = big.tile([HD, HL, Lk], BF16, tag="kT")
            qT = big.tile([HD, HL, Lq], BF16, tag="qT")
            vsb = big.tile([128, NKT, HL, HD + 1], BF16, tag="vsb")
            o2n = big.tile([HD, HL, Lq], BF16, tag="o2n")
            cst = ctx.enter_context(tc.tile_pool(name="cst", bufs=1))
            p96 = cst.tile([128, HD], BF16, tag="p96")
            bqh = cst.tile([128, HL], F32, tag="bqh")
            bkh = cst.tile([128, HL], F32, tag="bkh")
            bvb = cst.tile([128, DL], F32, tag="bvb")
            one96 = cst.tile([1, HD], MM_DT, tag="one96")

            nc.scalar.dma_start(p96[:], dp96[:])
            nc.scalar.dma_start(bqh[:], dbq[:])
            nc.scalar.dma_start(bkh[:], dbk[:])
            nc.scalar.dma_start(bvb[:], dbv[:])
            nc.scalar.dma_start(one96[:], done96[:])

            # ones column of v (softmax denominator accumulates in po[HD])
            nc.gpsimd.memset(vsb[:, :, :, HD : HD + 1], 1.0)

            # ---- q/k projections + rope ----
            # Two-stage software pipeline: the rope shuffle matmul (xs) lags
            # one head behind the projection (so the PE never waits on the
            # Scalar-engine xf copy), and the elementwise rope ops lag two
            # heads (so the xs PSUM slot never waits on the DVE FIFO).
            def proj_rope(src_dram, sin_dram, cos_dram, wT_tiles, bias_sb, dst, n_jt):
                pend_xs = [None]
                pend_alu = [None]

                def emit_xs(xf, sinD, cosD, out_ap, h):
                    xs = xs_ps.tile([HD, 512], F32, tag="xs")
                    nc.tensor.matmul(
                        xs[:], (p96[0:HD, :]), (xf[:]), start=True, stop=True
                    )
                    pend_alu[0] = (xf, xs, sinD, cosD, out_ap, h)

                def emit_alu(xf, xs, sinD, cosD, out_ap, h):
                    m2 = tmp.tile([HD, 512], BF16, tag="m2")
                    nc.vector.tensor_mul(m2[:], xs[:], sinD[:])
                    m1 = tmp.tile([HD, 512], BF16, tag="m1")
                    eng = nc.vector if h % 4 == 0 else nc.gpsimd
                    eng.tensor_mul(m1[:], xf[:], cosD[:])
                    nc.vector.tensor_add(out_ap, m1[:], m2[:])

                def step(nxt):
                    # advance the two-deep pipeline by one head
                    if pend_alu[0] is not None:
                        alu, pend_alu[0] = pend_alu[0], None
                    else:
                        alu = None
                    if pend_xs[0] is not None:
                        emit_xs(*pend_xs[0])  # sets pend_alu
                    pend_xs[0] = nxt
                    if alu is not None:
                        emit_alu(*alu)

                for jt in range(n_jt):
                    ts = slice(jt * 512, (jt + 1) * 512)
                    sinD = trig.tile([HD, 512], F32, tag="sin")
                    cosD = trig.tile([HD, 512], BF16, tag="cos")
                    nc.sync.dma_start(sinD[:], sin_dram[:, ts])
                    nc.sync.dma_start(cosD[:], cos_dram[:, ts])
                    stg = [stage.tile([128, 512], BF16, tag="stg", name=f"stg{c}") for c in range(NC6)]
                    for c in range(NC6):
                        nc.sync.dma_start(
                            stg[c][:], src_dram[c * 128 : (c + 1) * 128, ts]
                        )
                    for h in range(HL):
                        ps = pj_ps.tile([HD, 512], F32, tag="pj")
                        for c in range(NC6):
                            nc.tensor.matmul(
                                ps[:],
                                (wT_tiles[c][:, h * HD : (h + 1) * HD]),
                                (stg[c][:]),
                                start=(c == 0),
                                stop=(c == NC6 - 1),
                            )
                        xf = xf_p.tile([HD, 512], BF16, tag="xf")
                        nc.scalar.activation(
                            xf[:], ps[:], AF.Identity, bias=bias_sb[0:HD, h : h + 1]
                        )
                        step((xf, sinD, cosD, dst[:, h, ts], h))
                # drain the pipeline
                step(None)
                step(None)

            with ExitStack() as pctx:
                stage = pctx.enter_context(tc.tile_pool(name="stage", bufs=12))
                trig = pctx.enter_context(tc.tile_pool(name="trig", bufs=3))
                tmp = pctx.enter_context(tc.tile_pool(name="tmp", bufs=2))
                xf_p = pctx.enter_context(tc.tile_pool(name="xf", bufs=3))
                pj_ps = pctx.enter_context(
                    tc.tile_pool(name="pj_ps", bufs=2, space="PSUM")
                )
                vp_ps = pctx.enter_context(
                    tc.tile_pool(name="vp_ps", bufs=3, space="PSUM")
                )
                xs_ps = pctx.enter_context(
                    tc.tile_pool(name="xs_ps", bufs=3, space="PSUM")
                )

                with ExitStack() as wctx:
                    wq_p = wctx.enter_context(tc.tile_pool(name="wq", bufs=NC6))
                    wqT = [wq_p.tile([128, DL], BF16, tag="wq", name=f"wq{c}") for c in range(NC6)]
                    for c in range(NC6):
                        nc.scalar.dma_start(wqT[c][:], dwq[c * 128 : (c + 1) * 128, :])
                    proj_rope(dqT, dsq, dcq, wqT, bqh, qT, NJQ)

                with ExitStack() as wctx:
                    wk_p = wctx.enter_context(tc.tile_pool(name="wk", bufs=NC6))
                    wkT = [wk_p.tile([128, DL], BF16, tag="wk", name=f"wk{c}") for c in range(NC6)]
                    for c in range(NC6):
                        nc.sync.dma_start(wkT[c][:], dwk[c * 128 : (c + 1) * 128, :])
                    proj_rope(dkT, dsk, dck, wkT, bkh, kT, NJK)

                # ---- v projection (token-major, with bias add) ----
                with ExitStack() as wctx:
                    wv_p = wctx.enter_context(tc.tile_pool(name="wv", bufs=NC6))
                    wvT = [wv_p.tile([128, DL], BF16, tag="wv", name=f"wv{c}") for c in range(NC6)]
                    for c in range(NC6):
                        nc.sync.dma_start(wvT[c][:], dwv[c * 128 : (c + 1) * 128, :])
                    for jt in range(NJK):
                        ts = slice(jt * 512, (jt + 1) * 512)
                        stg = [stage.tile([128, 512], BF16, tag="stg", name=f"stg{c}") for c in range(NC6)]
                        for c in range(NC6):
                            nc.sync.dma_start(
                                stg[c][:], dvT[c * 128 : (c + 1) * 128, ts]
                            )
                        for sub in range(4):
                            kt = jt * 4 + sub
                            ps = vp_ps.tile([128, DL], F32, tag="vps")
                            for c in range(NC6):
                                nc.tensor.matmul(
                                    ps[:],
                                    (stg[c][:, sub * 128 : (sub + 1) * 128]),
                                    (wvT[c][:]),
                                    start=(c == 0),
                                    stop=(c == NC6 - 1),
                                )
                            nc.vector.scalar_tensor_tensor(
                                vsb[:, kt, :, 0:HD],
                                ps[:].rearrange("p (h d) -> p h d", h=HL),
                                0.0,
                                bvb[:].rearrange("p (h d) -> p h d", h=HL),
                                ALU.bypass,
                                ALU.add,
                            )

            # ---- attention + output projection ----
            # Flat software pipeline over all (jq, h, group) units: scores+exp
            # run one group ahead of attn@V, with no drain at head/jq
            # boundaries.  The softmax-denominator broadcast matmul is
            # deferred a couple of groups so the PE never waits on the DVE
            # normalize chain; jq0's output projection is interleaved into
            # jq1's score stream.
            with ExitStack() as actx:
                wo_p = actx.enter_context(tc.tile_pool(name="wo", bufs=4))
                woT = [wo_p.tile([128, D], BF16, tag="wo", name=f"wo{t}") for t in range(4)]
                for ht in range(4):
                    nc.sync.dma_start(woT[ht][:], dwo[ht * 128 : (ht + 1) * 128, :])
                s_ps = actx.enter_context(tc.tile_pool(name="s_ps", bufs=2, space="PSUM"))
                po_ps = actx.enter_context(tc.tile_pool(name="po_ps", bufs=2, space="PSUM"))
                bc_ps = actx.enter_context(tc.tile_pool(name="bc_ps", bufs=2, space="PSUM"))
                pt_p = actx.enter_context(tc.tile_pool(name="pt", bufs=3))
                nz_p = actx.enter_context(tc.tile_pool(name="nz", bufs=3))

                NG = NKT // GRP
                units = [(jq, h) for jq in range(NJQ) for h in range(HL)]
                groups = [(u, g) for u in range(len(units)) for g in range(NG)]
                po_of = {}
                pt_of = {}
                outproj_pend = []

                def emit_scores(u, g):
                    jq, h = units[u]
                    qs = slice(jq * 512, (jq + 1) * 512)
                    if g == 0:
                        po_of[u] = po_ps.tile([HD + 1, 512], F32, tag="po", name=f"po{u}")
                    sg = s_ps.tile([128, GRP * 512], F32, tag="sg")
                    for i in range(GRP):
                        kt = g * GRP + i
                        nc.tensor.matmul(
                            sg[:, i * 512 : (i + 1) * 512],
                            (kT[:, h, kt * 128 : (kt + 1) * 128]),
                            (qT[:, h, qs]),
                            start=True,
                            stop=True,
                        )
                    pt = pt_p.tile([128, GRP * 512], BF16, tag="pt")
                    nc.scalar.activation(pt[:], sg[:], AF.Exp, scale=SCALE)
                    pt_of[(u, g)] = pt

                def emit_attnv(u, g):
                    jq, h = units[u]
                    po = po_of[u]
                    pt = pt_of.pop((u, g))
                    for i in range(GRP):
                        kt = g * GRP + i
                        nc.tensor.matmul(
                            po[:],
                            (vsb[:, kt, h, :]),
                            (pt[:, i * 512 : (i + 1) * 512]),
                            start=(kt == 0),
                            stop=(kt == NKT - 1),
                        )
                    if g == NG - 1:
                        # start of normalize chain (DVE only; PE-independent)
                        s1 = nz_p.tile([1, 512], F32, tag="s1")
                        nc.vector.tensor_copy(s1[:], po[HD : HD + 1, :])
                        s1r = nz_p.tile([1, 512], F32, tag="s1r")
                        nc.vector.reciprocal_approx_fast(s1r[:], s1[:])
                        s1m = nz_p.tile([1, 512], MM_DT, tag="s1m")
                        nc.vector.tensor_copy(s1m[:], s1r[:])
                        po_of[u] = (po, s1m)

                def emit_normtail(u):
                    jq, h = units[u]
                    qs = slice(jq * 512, (jq + 1) * 512)
                    po, s1m = po_of.pop(u)
                    bcps = bc_ps.tile([HD, 512], F32, tag="bc", name="bcps")
                    nc.tensor.matmul(bcps[:], one96[:], s1m[:], start=True, stop=True)
                    bc = nz_p.tile([HD, 512], F32, tag="bc")
                    nc.vector.tensor_copy(bc[:], bcps[:])
                    nc.vector.tensor_mul(o2n[:, h, qs], po[0:HD, :], bc[:])

                def emit_outproj_e(jq, e):
                    qs = slice(jq * 512, (jq + 1) * 512)
                    pf = bc_ps.tile([128, 512], F32, tag="bc", name="pf")
                    for ht in range(4):
                        nc.tensor.matmul(
                            pf[:],
                            (woT[ht][0:HD, e * 128 : (e + 1) * 128]),
                            (o2n[:, ht, qs]),
                            start=(ht == 0),
                            stop=(ht == 3),
                        )
                    osb = nz_p.tile([128, 512], F32, tag="osb")
                    nc.vector.tensor_copy(osb[:], pf[:])
                    nc.gpsimd.dma_start(dout[e * 128 : (e + 1) * 128, qs], osb[:])

                LAG_BC = 3  # groups between last attn@V and the bcast matmul
                for idx in range(len(groups) + 1 + LAG_BC):
                    if idx < len(groups):
                        emit_scores(*groups[idx])
                    if 1 <= idx <= len(groups):
                        u, g = groups[idx - 1]
                        emit_attnv(u, g)
                        jq, h = units[u]
                        # interleave previous jq's output projection
                        if outproj_pend and g % 5 == 4:
                            emit_outproj_e(*outproj_pend.pop(0))
                    if idx - 1 - LAG_BC >= 0 and idx - 1 - LAG_BC < len(groups):
                        u, g = groups[idx - 1 - LAG_BC]
                        if g == NG - 1:
                            emit_normtail(u)
                            jq, h = units[u]
                            if h == HL - 1:
                                outproj_pend.extend(
                                    (jq, e) for e in range(NC6)
                                )
                # drain remaining output projections (jq1 tail)
                for jq, e in outproj_pend:
                    emit_outproj_e(jq, e)

    nc.compile()
    return nc


def _host_prep(inputs):
    """Build per-core input maps (numpy, bf16 for matmul operands)."""
    q = np.ascontiguousarray(np.asarray(inputs["query"], np.float32))
    k = np.ascontiguousarray(np.asarray(inputs["key"], np.float32))
    v = np.ascontiguousarray(np.asarray(inputs["value"], np.float32))
    cq = np.asarray(inputs["coords_query"], np.float32)
    ck = np.asarray(inputs["coords_key"], np.float32)
    Wq = np.asarray(inputs["Wq"], np.float32)
    Wk = np.asarray(inputs["Wk"], np.float32)
    Wv = np.asarray(inputs["Wv"], np.float32)
    Wo = np.asarray(inputs["Wo"], np.float32)
    bq = np.asarray(inputs["bq"], np.float32)
    bk = np.asarray(inputs["bk"], np.float32)
    bv = np.asarray(inputs["bv"], np.float32)

    inv_freq = (
        1.0 / (10000.0 ** (np.arange(16, dtype=np.float32) / np.float32(16.0)))
    ).astype(np.float32)
    # per-rope-row inverse frequency: row r -> axis r//32, freq inv_freq[r%16]
    row_freq = np.zeros(HD, np.float32)
    row_axis = np.zeros(HD, np.intp)
    for a in range(3):
        for h2 in range(2):
            for j in range(16):
                r = a * 32 + h2 * 16 + j
                row_freq[r] = inv_freq[j]
                row_axis[r] = a
    p96 = np.zeros((128, HD), np.float32)
    for a in range(3):
        for j in range(16):
            p96[a * 32 + 16 + j, a * 32 + j] = -1.0
            p96[a * 32 + j, a * 32 + 16 + j] = 1.0

    def trig_tables(c):  # [L, 3] -> sin [HD, L] fp32, cos [HD, L] bf16
        theta = row_freq[:, None] * c.T[row_axis, :]  # [HD, L]
        return (
            np.sin(theta).astype(np.float32),
            np.cos(theta).astype(NP_BF16),
        )

    in_maps = []
    for c in range(N_CORES):
        b, hg = c // 2, c % 2
        dsl = slice(hg * DL, (hg + 1) * DL)
        wo_pad = np.zeros((4 * 128, D), np.float32)
        woT_full = np.ascontiguousarray(Wo.T)
        for ht in range(4):
            wo_pad[ht * 128 : ht * 128 + HD, :] = woT_full[
                hg * DL + ht * HD : hg * DL + (ht + 1) * HD, :
            ]
        bqh = np.zeros((128, HL), np.float32)
        bkh = np.zeros((128, HL), np.float32)
        for h in range(HL):
            bqh[:HD, h] = bq[hg * DL + h * HD : hg * DL + (h + 1) * HD]
            bkh[:HD, h] = bk[hg * DL + h * HD : hg * DL + (h + 1) * HD]
        bvb = np.tile(bv[dsl][None, :], (128, 1)).astype(np.float32)
        sinq, cosq = trig_tables(cq[b])
        sink, cosk = trig_tables(ck[b])
        in_maps.append(
            {
                "queryT": np.ascontiguousarray(q[b].T).astype(NP_BF16),
                "keyT": np.ascontiguousarray(k[b].T).astype(NP_BF16),
                "valueT": np.ascontiguousarray(v[b].T).astype(NP_BF16),
                "sinq": sinq,
                "cosq": cosq,
                "sink": sink,
                "cosk": cosk,
                "wqT": np.ascontiguousarray(Wq[dsl, :].T).astype(NP_BF16),
                "wkT": np.ascontiguousarray(Wk[dsl, :].T).astype(NP_BF16),
                "wvT": np.ascontiguousarray(Wv[dsl, :].T).astype(NP_BF16),
                "woT": wo_pad.astype(NP_BF16),
                "bqh": bqh,
                "bkh": bkh,
                "bvb": bvb,
                "p96": p96.astype(NP_BF16),
                "one96": np.ones((1, HD), np.float32),
            }
        )
    return in_maps


def _run(inputs, trace=False):
    nc = build_program()
    in_maps = _host_prep(inputs)
    res = bass_utils.run_bass_kernel_spmd(
        nc, in_maps, core_ids=list(range(N_CORES)), trace=trace
    )
    bo = np.asarray(inputs["bo"], np.float32)
    out = np.empty((B, Lq, D), np.float32)
    for b in range(B):
        acc = res.results[2 * b]["outT"] + res.results[2 * b + 1]["outT"]
        out[b] = acc.T + bo
    return out, res


def kernel(**inputs) -> np.ndarray:
    out, _ = _run(inputs, trace=False)
    return out
